# revision 16
# baseline (speedup 1.0000x reference)
"""LlamaAttention (B=1, S=4096, H=1024, NH=16, NKV=4, HD=64) on 8 TRN2 NeuronCores.

Sharding: tensor-parallel over heads. Core c owns query heads {2c, 2c+1} and
their shared KV head g=c//2, processes the full 4096-token sequence:
  - projections for its heads (q/k/v) from a host-pre-transposed bf16 hidden^T
  - RoPE fused into the PSUM->SBUF eviction of q^T / k^T
  - causal flash-style attention with scores computed transposed
    (S^T[key, q] = k^T.T @ q^T), exp on ScalarE batched over 3 PSUM banks,
    ones-column in V for softmax denominators, block-skipped causal structure
    plus one [128,128] triangular bf16 mask for diagonal blocks
  - local output projection against its 128 columns of Wo -> per-core partial
    o (full [4096, 1024]); the 8 partials are summed on the host (unshard of a
    contraction-sharded output).
"""
import numpy as np
import ml_dtypes

import axon_compat
axon_compat.install()

import concourse.bass as bass
import concourse.tile as tile
from concourse import mybir
from concourse.bass_utils import run_bass_kernel_spmd

BF16 = mybir.dt.bfloat16
F32 = mybir.dt.float32
AF = mybir.ActivationFunctionType

S, H, NH, NKV, HD = 4096, 1024, 16, 4, 64
NC = 8
ROPE_THETA = 10000.0

_nc_cache = {}


def build_nc():
    if "nc" in _nc_cache:
        return _nc_cache["nc"]
    nc = bass.Bass("TRN2", target_bir_lowering=False, debug=False, num_devices=NC)

    hT = nc.dram_tensor("hT", [H, S], BF16, kind="ExternalInput").ap()
    wq = nc.dram_tensor("wq", [H, 128], BF16, kind="ExternalInput").ap()
    wkv = nc.dram_tensor("wkv", [H, 128], BF16, kind="ExternalInput").ap()
    wo = nc.dram_tensor("wo", [128, H], BF16, kind="ExternalInput").ap()
    cosD = nc.dram_tensor("cosD", [128, S], F32, kind="ExternalInput").ap()
    sinS = nc.dram_tensor("sinS", [128, S], F32, kind="ExternalInput").ap()
    tri = nc.dram_tensor("tri", [128, 128], BF16, kind="ExternalInput").ap()
    ident = nc.dram_tensor("ident", [128, 128], BF16, kind="ExternalInput").ap()
    identf = nc.dram_tensor("identf", [1, 1], F32, kind="ExternalInput").ap()
    opart = nc.dram_tensor("opart", [S, H], F32, kind="ExternalOutput").ap()

    with tile.TileContext(nc) as tc:
        with (
            tc.tile_pool(name="persist", bufs=1) as persist,
            tc.tile_pool(name="consts", bufs=1) as consts,
        ):
            # Persistent SBUF tensors
            qT = persist.tile([128, S], BF16)       # rows 0-63 head A, 64-127 head B
            kTd = persist.tile([128, S], BF16)      # k^T duplicated on both halves
            vT = persist.tile([64, S], BF16)
            vaug = persist.tile([128, 32 * 66], BF16)  # per key-block [64 V | 1 | pad]
            abuf = persist.tile([128, S], BF16)     # unnormalized attn^T numerators
            dbufA = persist.tile([1, S], F32)       # softmax denominators head A
            dbufB = persist.tile([1, S], F32)       # softmax denominators head B
            dT_sb = persist.tile([128, 64], F32)    # 1/denom, transposed, per q-chunk
            tri_sb = consts.tile([128, 128], BF16)
            id_sb = consts.tile([128, 128], BF16)
            idf_sb = consts.tile([1, 1], F32)
            nc.sync.dma_start(tri_sb[:], tri[:])
            nc.sync.dma_start(id_sb[:], ident[:])
            nc.vector.memset(vaug[:], 0.0)
            nc.sync.dma_start(idf_sb[:], identf[:])

            # ---------------- Stage A: projections + RoPE + V layout -------
            with (
                tc.tile_pool(name="hT", bufs=1) as hpool,
                tc.tile_pool(name="w", bufs=1) as wpool,
                tc.tile_pool(name="trig", bufs=1) as trig,
                tc.tile_pool(name="pjq", bufs=2, space="PSUM") as pjq,
                tc.tile_pool(name="pjkv", bufs=2, space="PSUM") as pjkv,
                tc.tile_pool(name="ptv", bufs=2, space="PSUM") as ptv,
                tc.tile_pool(name="ropetmp", bufs=2) as ropetmp,
            ):
                wq_sb = wpool.tile([128, 8 * 128], BF16, tag="w")
                wkv_sb = wpool.tile([128, 8 * 128], BF16, tag="w2")
                cos_sb = trig.tile([128, S], F32, tag="cos")
                sin_sb = trig.tile([128, S], F32, tag="sin")
                for k in range(8):
                    nc.sync.dma_start(wq_sb[:, k * 128:(k + 1) * 128],
                                      wq[k * 128:(k + 1) * 128, :])
                    nc.sync.dma_start(wkv_sb[:, k * 128:(k + 1) * 128],
                                      wkv[k * 128:(k + 1) * 128, :])
                nc.sync.dma_start(cos_sb[:], cosD[:])
                nc.sync.dma_start(sin_sb[:], sinS[:])

                hts = []
                for k in range(8):
                    ht_k = hpool.tile([128, S], BF16, tag=f"ht{k}")
                    nc.sync.dma_start(ht_k[:], hT[k * 128:(k + 1) * 128, :])
                    hts.append(ht_k)

                def rope_evict(psum, rows, row0, out, ocol, ncol, cs, ss):
                    # out[row0:row0+rows, ocol:ocol+ncol] =
                    #   psum[0:rows] * cos + swapped(psum) * sinSigned
                    m = ropetmp.tile([rows, 512], F32, tag="m")
                    n = ropetmp.tile([rows, 512], F32, tag="n")
                    nc.vector.tensor_mul(m[:, :ncol], psum[0:rows, :ncol],
                                         cs[0:rows, ocol:ocol + ncol])
                    for h0 in range(0, rows, 64):
                        nc.vector.tensor_mul(
                            n[h0:h0 + 32, :ncol], psum[h0 + 32:h0 + 64, :ncol],
                            ss[h0:h0 + 32, ocol:ocol + ncol])
                        nc.vector.tensor_mul(
                            n[h0 + 32:h0 + 64, :ncol], psum[h0:h0 + 32, :ncol],
                            ss[h0 + 32:h0 + 64, ocol:ocol + ncol])
                    nc.vector.tensor_add(out[row0:row0 + rows, ocol:ocol + ncol],
                                         m[:, :ncol], n[:, :ncol])

                NCH = S // 512
                for n_i in range(NCH):
                    c0 = n_i * 512
                    pq = pjq.tile([128, 512], F32)
                    pkv = pjkv.tile([128, 512], F32)
                    for k in range(8):
                        nc.tensor.matmul(pq[:], wq_sb[:, k * 128:(k + 1) * 128],
                                         hts[k][:, c0:c0 + 512],
                                         start=(k == 0), stop=(k == 7))
                    for k in range(8):
                        nc.tensor.matmul(pkv[:], wkv_sb[:, k * 128:(k + 1) * 128],
                                         hts[k][:, c0:c0 + 512],
                                         start=(k == 0), stop=(k == 7))
                    rope_evict(pq, 128, 0, qT, c0, 512, cos_sb, sin_sb)
                    rope_evict(pkv, 64, 0, kTd, c0, 512, cos_sb, sin_sb)
                    nc.vector.tensor_copy(vT[:, c0:c0 + 512], pkv[64:128, :])
                # duplicate k^T to partitions 64-127
                nc.vector.tensor_copy(kTd[64:128, :], kTd[0:64, :])
                # V natural layout via PE transpose, plus ones column
                for kb in range(32):
                    pv = ptv.tile([128, 64], BF16)
                    nc.tensor.transpose(pv[:], vT[:, kb * 128:(kb + 1) * 128],
                                        id_sb[0:64, 0:64])
                    nc.vector.tensor_copy(vaug[:, kb * 66:kb * 66 + 64], pv[:])
                    nc.vector.memset(vaug[:, kb * 66 + 64:kb * 66 + 65], 1.0)

            # ---------------- Stage B: attention ---------------------------
            with (
                tc.tile_pool(name="st", bufs=2, space="PSUM") as stp,
                tc.tile_pool(name="attn", bufs=1, space="PSUM") as attnp,
                tc.tile_pool(name="est", bufs=2) as estp,
            ):
                # flat work list: (super, kb, head) with causal F
                items = []
                for s_i in range(8):
                    qo = 512 * s_i
                    for kb in range(4 * s_i + 4):
                        qstart = max(qo, 128 * kb)
                        F = qo + 512 - qstart
                        for head in range(2):
                            items.append((s_i, kb, head, qstart, F))
                assert len(items) % 3 == 0

                st_t = None
                est_t = None
                pend = []  # (item, slot_off, est_tile)
                attn_ps = {}
                for idx, (s_i, kb, head, qstart, F) in enumerate(items):
                    slot = idx % 3
                    if slot == 0:
                        st_t = stp.tile([128, 1536], F32)
                        est_t = estp.tile([128, 1536], BF16)
                    off = slot * 512
                    r0 = 64 * head
                    nc.tensor.matmul(
                        st_t[:, off:off + F],
                        kTd[r0:r0 + 64, kb * 128:(kb + 1) * 128],
                        qT[r0:r0 + 64, qstart:qstart + F],
                        start=True, stop=True)
                    pend.append(((s_i, kb, head, qstart, F), off, st_t, est_t))
                    if slot == 2:
                        nc.scalar.activation(est_t[:], st_t[:], AF.Exp, scale=0.125)
                        for (it, o2, _st2, est2) in pend:
                            s2, kb2, hd2, qs2, F2 = it
                            if 128 * kb2 >= 512 * s2:  # diagonal block
                                nc.vector.tensor_mul(
                                    est2[:, o2:o2 + 128], est2[:, o2:o2 + 128],
                                    tri_sb[:])
                            key = (s2, hd2)
                            if key not in attn_ps:
                                attn_ps[key] = attnp.tile(
                                    [65, 512], F32, tag=f"at{hd2}",
                                    name=f"attn_s{s2}_h{hd2}")
                            qoff = qs2 - 512 * s2
                            nc.tensor.matmul(
                                attn_ps[key][:, qoff:qoff + F2],
                                vaug[:, kb2 * 66:kb2 * 66 + 65],
                                est2[:, o2:o2 + F2],
                                start=(kb2 == 0), stop=(kb2 == 4 * s2 + 3))
                            if kb2 == 4 * s2 + 3:
                                # super finished for this head: evacuate
                                # numerators (bf16) and denominators (f32)
                                ap = attn_ps.pop(key)
                                qo2 = 512 * s2
                                nc.vector.tensor_copy(
                                    abuf[64 * hd2:64 * hd2 + 64, qo2:qo2 + 512],
                                    ap[0:64, :])
                                db = dbufA if hd2 == 0 else dbufB
                                nc.vector.tensor_copy(
                                    db[0:1, qo2:qo2 + 512], ap[64:65, :])
                        pend = []

            # ---------------- Stage B2: transpose + invert denominators ----
            with tc.tile_pool(name="pdt", bufs=4, space="PSUM") as pdtp:
                for qc in range(32):
                    for hd, db in ((0, dbufA), (1, dbufB)):
                        pd = pdtp.tile([128, 1], F32, tag="pd", name=f"pd_{qc}_{hd}")
                        nc.tensor.transpose(
                            pd[:], db[0:1, qc * 128:(qc + 1) * 128], idf_sb[:])
                        nc.vector.reciprocal(
                            dT_sb[:, 2 * qc + hd:2 * qc + hd + 1], pd[:])

            # ---------------- Stage C: output projection -------------------
            # o[q,:] = numer_A[:,q]/den_A[q] @ woA + numer_B[:,q]/den_B[q] @ woB
            # normalization folded in as per-partition (per-q) scales.
            with (
                tc.tile_pool(name="wo", bufs=1) as wop,
                tc.tile_pool(name="poa", bufs=2, space="PSUM") as popa,
                tc.tile_pool(name="pob", bufs=2, space="PSUM") as popb,
                tc.tile_pool(name="ob", bufs=4) as obp,
            ):
                wo_sb = wop.tile([128, H], BF16)
                nc.sync.dma_start(wo_sb[:], wo[:])
                for qc in range(32):
                    qs = slice(qc * 128, (qc + 1) * 128)
                    for hb in range(2):
                        hs = slice(hb * 512, (hb + 1) * 512)
                        poa = popa.tile([128, 512], F32)
                        pob = popb.tile([128, 512], F32)
                        nc.tensor.matmul(poa[:], abuf[0:64, qs],
                                         wo_sb[0:64, hs], start=True, stop=True)
                        nc.tensor.matmul(pob[:], abuf[64:128, qs],
                                         wo_sb[64:128, hs], start=True, stop=True)
                        t1 = obp.tile([128, 512], F32, tag="t1")
                        nc.vector.tensor_scalar_mul(
                            t1[:], poa[:], dT_sb[:, 2 * qc:2 * qc + 1])
                        ob = obp.tile([128, 512], F32, tag="ob")
                        nc.vector.scalar_tensor_tensor(
                            ob[:], pob[:], dT_sb[:, 2 * qc + 1:2 * qc + 2], t1[:],
                            op0=mybir.AluOpType.mult, op1=mybir.AluOpType.add)
                        nc.sync.dma_start(opart[qs, hs], ob[:])

    axon_compat.split_multiwait(nc)
    _nc_cache["nc"] = nc
    return nc


def _host_prep(hidden_states, position_ids, Wq, Wk, Wv, Wo):
    bf16 = ml_dtypes.bfloat16
    hTb = np.ascontiguousarray(hidden_states[0].T).astype(bf16)  # [H, S]

    pos = np.asarray(position_ids[0]).astype(np.float32)  # [S]
    inv = 1.0 / (ROPE_THETA ** (np.arange(0, HD, 2, dtype=np.float32) / HD))
    freqs = pos[:, None] * inv[None, :]                    # [S, 32]
    emb = np.concatenate([freqs, freqs], axis=1)           # [S, 64]
    cos = np.cos(emb).astype(np.float32)                   # [S, 64]
    sin = np.sin(emb).astype(np.float32)
    cosD = np.ascontiguousarray(np.tile(cos.T, (2, 1)))    # [128, S]
    sinT = sin.T                                           # [64, S]
    sinSg = np.concatenate([-sinT[:32], sinT[32:]], axis=0)
    sinS = np.ascontiguousarray(np.tile(sinSg, (2, 1)))    # [128, S]

    i = np.arange(128)
    tri = (i[:, None] <= np.arange(128)[None, :]).astype(bf16)
    ident = np.eye(128, dtype=bf16)

    in_maps = []
    for c in range(NC):
        g = c // 2
        wq_s = np.ascontiguousarray(Wq[128 * c:128 * (c + 1), :].T).astype(bf16)
        wkv_s = np.ascontiguousarray(
            np.concatenate([Wk[64 * g:64 * (g + 1), :],
                            Wv[64 * g:64 * (g + 1), :]], axis=0).T).astype(bf16)
        wo_s = np.ascontiguousarray(Wo[:, 128 * c:128 * (c + 1)].T).astype(bf16)
        in_maps.append({
            "hT": hTb, "wq": wq_s, "wkv": wkv_s, "wo": wo_s,
            "cosD": cosD, "sinS": sinS, "tri": tri, "ident": ident,
            "identf": np.eye(1, dtype=np.float32),
        })
    return in_maps


def kernel(hidden_states, attention_mask, position_ids, Wq, Wk, Wv, Wo):
    nc = build_nc()
    in_maps = _host_prep(hidden_states, position_ids, Wq, Wk, Wv, Wo)
    res = run_bass_kernel_spmd(nc, in_maps, list(range(NC)))
    out = np.zeros((S, H), dtype=np.float32)
    for c in range(NC):
        out += res.results[c]["opart"]
    return out[None].astype(np.float32)
